# revision 18
# baseline (speedup 1.0000x reference)
"""Trainium2 Bass kernel for nn_Attention_33457795236557.

Math (B,H,S,D,HID = 2,4,512,16,32):
  qp = q@Wq+bq ; kp = k@Wk+bk ; vp = v@Wv+bv
  term1[i,j] = relu(qp_i@A + kp_j@Bm + b1) @ N2        (A=N1[:D], Bm=N1[D:])
  term2[i,j] = relu(kp_j@A + qp_i@Bm + b1) @ N2        (symmetrized, swapaxes)
  logits = term1 + term2 (+2*b2, const -> softmax-invariant) + mask*-1e9
  attn = softmax(logits, axis=-1) ; out = (attn @ vp) @ Wo + bo

Sharding: data-parallel over (b,h) -> 8 cores, one (b,h) pair each.

Device strategy per core (S=512 queries, tiled 4x128):
  Host folds all weight products:  hqA = q_aug @ CQ1, hkB' = k_aug @ CB1 (+b1),
  etc., with q_aug = [q | 1].  For each 128-query tile, queries are processed
  in blocks of 4; a block's relu input lives in a [128, 512] SBUF tile with
  partition p = i_local*32 + f (4 queries x 32 hidden), built by one
  DVE tensor_scalar (or ACT relu) op:  R = relu(HkRep + hq_col).
  HkRep[p, j] = hk'[j, f(p)] is precomputed once per core via one PE matmul.
  The hidden contraction R -> logits uses one fp32r PE matmul per relu tile
  with a zero-padded shifted block-diagonal N2 weight (lhsT [128,128], only
  columns 4b'..4b'+3 nonzero), all 64 matmuls of a query tile accumulating
  into one full [128, 512] PSUM bank (PE time is stream-column bound, so the
  zero padding is free).
  Softmax: ACT exp straight off PSUM logits (|logit| <= ~6, so the max
  subtraction is skipped -- softmax is shift invariant and fp32 exp cannot
  overflow) with accum_out producing the row sums; DVE reciprocal; DVE
  tensor_scalar normalize.  Output: PE transposes of the unnormalized attn,
  then attn @ (v_aug @ [Wv@Wo; bv@Wo+bo]) accumulated over 4 key chunks on
  PE, scaled by 1/rowsum at the end (bias terms fold through because
  normalized attn rows sum to 1).
"""

from contextlib import ExitStack

import numpy as np

import concourse.tile as tile
from concourse import bacc, mybir
from concourse.bass_utils import run_bass_kernel_spmd

B, H, S, D, HID = 2, 4, 512, 16, 32
NCORES = 8
F32 = mybir.dt.float32
F32R = mybir.dt.float32r

# Fraction of relu tiles routed to the scalar (ACT) engine: indices with
# (ridx % ACT_DEN) < ACT_MOD go to ACT, rest to DVE.
ACT_MOD = 1
ACT_DEN = 3
# engine toggles for small ops (tuned via TimelineSim)
HQCOL_ENG = "vector"   # gpsimd | vector
AT_ENG = "scalar"      # scalar | vector
HKREP_ENG = "scalar"   # scalar | vector
# dtype of the relu tiles + N2 weights feeding the PE contraction:
# "f32r" (full precision path, HW err ~1.6e-4) or "bf16" (faster DVE 4x mode)
R_DTYPE = "f32r"

LAST_RESULTS = None


def _build_program(with_mask: bool):
    nc = bacc.Bacc("TRN2", target_bir_lowering=False, debug=False,
                   enable_asserts=False)

    # Column layout of the packed f32 constants tensor [17, 1872]
    # (one hot DMA; ident and the fp32r N2 block-diagonals ship separately).
    COLS = dict(qT=(0, 512), kT=(512, 1024), vT=(1024, 1536),
                cq1=(1536, 1568), cq2=(1568, 1600),
                cb1=(1600, 1728), cb2=(1728, 1856), wvo=(1856, 1872))
    W = 1872
    packed = nc.dram_tensor("packed", [D + 1, W], F32,
                            kind="ExternalInput").ap()
    ident_d = nc.dram_tensor("ident", [128, 128], F32,
                             kind="ExternalInput").ap()
    rdt = F32R if R_DTYPE == "f32r" else mybir.dt.bfloat16
    n2shift = nc.dram_tensor("n2shift", [128, 32 * 128], rdt,
                             kind="ExternalInput").ap()
    if with_mask:
        maskeff = nc.dram_tensor("maskeff", [S, S], F32,
                                 kind="ExternalInput").ap()

    out_d = nc.dram_tensor("out", [S, D], F32, kind="ExternalOutput").ap()
    attn_d = nc.dram_tensor("attn", [S, S], F32, kind="ExternalOutput").ap()

    with tile.TileContext(nc) as tc, ExitStack() as ctx:
        consts = ctx.enter_context(tc.tile_pool(name="consts", bufs=1))
        hkrep_p = ctx.enter_context(tc.tile_pool(name="hkrep", bufs=1))
        hqt_p = ctx.enter_context(tc.tile_pool(name="hqt", bufs=1))
        hqcol_p = ctx.enter_context(tc.tile_pool(name="hqcol", bufs=3))
        r_p = ctx.enter_context(tc.tile_pool(name="r", bufs=16))
        attn_p = ctx.enter_context(tc.tile_pool(name="attn", bufs=2))
        attnT_p = ctx.enter_context(tc.tile_pool(name="attnT", bufs=5))
        stats_p = ctx.enter_context(tc.tile_pool(name="stats", bufs=4))
        outsb_p = ctx.enter_context(tc.tile_pool(name="outsb", bufs=2))
        vpo_p = ctx.enter_context(tc.tile_pool(name="vpo", bufs=1))

        pre_ps = ctx.enter_context(
            tc.tile_pool(name="pre_ps", bufs=1, space="PSUM"))
        logit_ps = ctx.enter_context(
            tc.tile_pool(name="logit_ps", bufs=4, space="PSUM"))
        t_ps = ctx.enter_context(
            tc.tile_pool(name="t_ps", bufs=2, space="PSUM"))
        o_ps = ctx.enter_context(
            tc.tile_pool(name="o_ps", bufs=1, space="PSUM"))

        # ---- load constants/inputs ----
        big = consts.tile([D + 1, W], F32, tag="packed")
        nc.sync.dma_start(big[:], packed[:])
        # N2 block-diagonals in 4 chunks so the first contraction matmuls
        # don't wait on the full 2MB transfer.
        n2_chunks = []
        for ch in range(4):
            t = consts.tile([128, 8 * 128], rdt, tag=f"n2_{ch}")
            nc.sync.dma_start(t[:], n2shift[:, 1024 * ch:1024 * (ch + 1)])
            n2_chunks.append(t)
        ident_s = consts.tile([128, 128], F32, tag="ident")
        nc.sync.dma_start(ident_s[:], ident_d[:])

        def cview(name):
            lo, hi = COLS[name]
            return big[:, lo:hi]

        qT_s, kT_s, vT_s = cview("qT"), cview("kT"), cview("vT")
        cq_s = [cview("cq1"), cview("cq2")]
        cb_s = [cview("cb1"), cview("cb2")]
        wvo_s = cview("wvo")

        # ---- prework: vpo' = v_aug @ [WvWo; bvWo+bo], per 128-key chunk ----
        vpo_s = vpo_p.tile([128, 4 * D], F32)
        for c in range(4):
            ps = pre_ps.tile([128, D], F32, tag="pre")
            nc.tensor.matmul(ps[:], vT_s[:, 128 * c:128 * (c + 1)], wvo_s[:],
                             start=True, stop=True, skip_group_check=True)
            nc.vector.tensor_copy(vpo_s[:, D * c:D * (c + 1)], ps[:])

        # hqT[term] = (q-side hidden)^T : [32, 512]
        hqt_s = []
        for term in range(2):
            ps = pre_ps.tile([HID, S], F32, tag="pre")
            nc.tensor.matmul(ps[:], cq_s[term][:], qT_s[:],
                             start=True, stop=True, skip_group_check=True)
            t = hqt_p.tile([HID, S], F32, tag=f"hqt{term}")
            nc.vector.tensor_copy(t[:], ps[:])
            hqt_s.append(t)

        # HkRep[term][p, j] = hk'[j, f(p)] replicated over i_local: [128, 512]
        hkrep_s = []
        for term in range(2):
            ps = pre_ps.tile([128, S], F32, tag="pre")
            nc.tensor.matmul(ps[:], cb_s[term][:], kT_s[:],
                             start=True, stop=True, skip_group_check=True)
            t = hkrep_p.tile([128, S], rdt if R_DTYPE == "bf16" else F32,
                             tag=f"hkrep{term}")
            if HKREP_ENG == "scalar":
                nc.scalar.copy(t[:], ps[:])
            else:
                nc.vector.tensor_copy(t[:], ps[:])
            hkrep_s.append(t)

        # ---- main loop over 4 query tiles (software-pipelined emission:
        # tile tq's relu+matmul stream first, then tile tq-1's softmax/out
        # chain, so the ACT exp never convoys the next tile's relu work) ----
        def emit_softmax_out(tq, logits):
            if with_mask:
                msk = attn_p.tile([128, S], F32, tag="mask")
                nc.sync.dma_start(
                    msk[:], maskeff[128 * tq:128 * (tq + 1), :])
                masked = attn_p.tile([128, S], F32, tag="masked")
                nc.vector.tensor_add(out=masked[:], in0=logits[:],
                                     in1=msk[:])
                lsrc = masked
            else:
                lsrc = logits
            attn_un = attn_p.tile([128, S], F32, tag="attn_un")
            sumexp = stats_p.tile([128, 1], F32, tag="sumexp")
            nc.scalar.activation(attn_un[:], lsrc[:],
                                 mybir.ActivationFunctionType.Exp,
                                 bias=0.0, scale=1.0,
                                 accum_out=sumexp[:])
            rinv = stats_p.tile([128, 1], F32, tag="rinv")
            nc.vector.reciprocal(rinv[:], sumexp[:])
            attn_n = attn_p.tile([128, S], F32, tag="attn_n")
            nc.vector.tensor_scalar_mul(out=attn_n[:], in0=attn_un[:],
                                        scalar1=rinv[:])
            nc.sync.dma_start(attn_d[128 * tq:128 * (tq + 1), :], attn_n[:])

            # out = (attn @ vpo') * rinv
            ops = o_ps.tile([128, D], F32)
            for c in range(4):
                tp = t_ps.tile([128, 128], F32)
                nc.tensor.matmul(tp[:], attn_un[:, 128 * c:128 * (c + 1)],
                                 ident_s[:], is_transpose=True,
                                 skip_group_check=True)
                aT = attnT_p.tile([128, 128], F32, tag="aT")
                if AT_ENG == "scalar":
                    nc.scalar.copy(aT[:], tp[:])
                else:
                    nc.vector.tensor_copy(aT[:], tp[:])
                nc.tensor.matmul(ops[:], aT[:], vpo_s[:, D * c:D * (c + 1)],
                                 start=(c == 0), stop=(c == 3),
                                 skip_group_check=True)
            osb = outsb_p.tile([128, D], F32)
            nc.scalar.mul(osb[:], ops[:], rinv[:])
            nc.sync.dma_start(out_d[128 * tq:128 * (tq + 1), :], osb[:])

        ridx = 0
        pending = None
        for tq in range(4):
            # hq columns for this tile: [128, 32]; col b' holds
            # hq[128*tq + 4*b' + i_l, f] at partition p = i_l*32 + f
            hqcol = []
            for term in range(2):
                hc = hqcol_p.tile([128, 32], F32, tag=f"hqcol{term}")
                src3 = hqt_s[term][:, 128 * tq:128 * (tq + 1)].rearrange(
                    "p (b il) -> p b il", il=4)
                cp = (nc.gpsimd.tensor_copy if HQCOL_ENG == "gpsimd"
                      else nc.vector.tensor_copy)
                for il in range(4):
                    cp(hc[32 * il:32 * (il + 1), :], src3[:, :, il])
                hqcol.append(hc)

            logits = logit_ps.tile([128, S], F32)
            for bprime in range(32):
                for term in range(2):
                    r = r_p.tile([128, S], rdt, tag="r")
                    col = hqcol[term][:, bprime:bprime + 1]
                    if ((ridx * ACT_MOD) % ACT_DEN) < ACT_MOD:
                        nc.scalar.activation(
                            r[:], hkrep_s[term][:],
                            mybir.ActivationFunctionType.Relu,
                            bias=col, scale=1.0)
                    else:
                        nc.vector.tensor_scalar(
                            out=r[:], in0=hkrep_s[term][:],
                            scalar1=col, scalar2=0.0,
                            op0=mybir.AluOpType.add,
                            op1=mybir.AluOpType.max)
                    ridx += 1
                    nc.tensor.matmul(
                        logits[:],
                        n2_chunks[bprime // 8][:, 128 * (bprime % 8):
                                               128 * (bprime % 8 + 1)],
                        r[:],
                        start=(bprime == 0 and term == 0),
                        stop=(bprime == 31 and term == 1),
                        skip_group_check=True)

            if pending is not None:
                emit_softmax_out(*pending)
            pending = (tq, logits)
        emit_softmax_out(*pending)

    nc.compile()
    return nc


def _host_fold(inputs):
    """Fold weights on host; returns dict of per-core-constant arrays."""
    f = lambda x: np.asarray(x, dtype=np.float32)
    Wq, bq = f(inputs["Wq"]), f(inputs["bq"])
    Wk, bk = f(inputs["Wk"]), f(inputs["bk"])
    Wv, bv = f(inputs["Wv"]), f(inputs["bv"])
    Wo, bo = f(inputs["Wo"]), f(inputs["bo"])
    N1, b1 = f(inputs["N1"]), f(inputs["b1"])
    N2 = f(inputs["N2"])
    A, Bm = N1[:D], N1[D:]

    def aug(W, b):
        return np.vstack([W, b[None, :]]).astype(np.float32)

    cq1 = aug(Wq @ A, bq @ A)                    # term1 q-side  [17, 32]
    cq2 = aug(Wq @ Bm, bq @ Bm)                  # term2 q-side
    cb1 = np.tile(aug(Wk @ Bm, bk @ Bm + b1), (1, 4))   # term1 k-side [17,128]
    cb2 = np.tile(aug(Wk @ A, bk @ A + b1), (1, 4))     # term2 k-side
    wvo = aug(Wv @ Wo, bv @ Wo + bo)             # [17, 16]

    n2shift = np.zeros((32, 128, 128), np.float32)
    n2f = N2[:, 0]
    for bp in range(32):
        for il in range(4):
            for ff in range(HID):
                n2shift[bp, il * 32 + ff, 4 * bp + il] = n2f[ff]

    return dict(cq1=cq1, cq2=cq2, cb1=np.ascontiguousarray(cb1),
                cb2=np.ascontiguousarray(cb2), wvo=wvo,
                n2shift=np.ascontiguousarray(
                    n2shift.transpose(1, 0, 2).reshape(128, 32 * 128)),
                ident=np.eye(128, dtype=np.float32))


def _make_in_maps(inputs, with_mask):
    consts = _host_fold(inputs)
    q = np.asarray(inputs["q"], dtype=np.float32)
    k = np.asarray(inputs["k"], dtype=np.float32)
    v = np.asarray(inputs["v"], dtype=np.float32)
    mask = np.asarray(inputs["mask"], dtype=np.float32)
    b2 = np.asarray(inputs["b2"], dtype=np.float32)

    ones = np.ones((1, S), np.float32)
    W = 1872
    base = np.zeros((17, W), np.float32)
    base[:, 1536:1568] = consts["cq1"]
    base[:, 1568:1600] = consts["cq2"]
    base[:, 1600:1728] = consts["cb1"]
    base[:, 1728:1856] = consts["cb2"]
    base[:, 1856:1872] = consts["wvo"]

    in_maps = []
    for c in range(NCORES):
        b, h = c // H, c % H
        p = base.copy()
        p[:, 0:512] = np.vstack([q[b, h].T, ones])
        p[:, 512:1024] = np.vstack([k[b, h].T, ones])
        p[:, 1024:1536] = np.vstack([v[b, h].T, ones])
        n2 = consts["n2shift"]
        if R_DTYPE == "bf16":
            import ml_dtypes
            n2 = n2.astype(ml_dtypes.bfloat16)
        m = {"packed": p, "n2shift": n2, "ident": consts["ident"]}
        if with_mask:
            m["maskeff"] = np.ascontiguousarray(
                mask[b, 0] * np.float32(-1e9) + 2.0 * b2[0])
        in_maps.append(m)
    return in_maps


_PROGRAM_CACHE = {}


def kernel(**inputs):
    global LAST_RESULTS
    mask = np.asarray(inputs["mask"], dtype=np.float32)
    with_mask = bool(np.any(mask))

    if with_mask not in _PROGRAM_CACHE:
        _PROGRAM_CACHE[with_mask] = _build_program(with_mask)
    nc = _PROGRAM_CACHE[with_mask]

    in_maps = _make_in_maps(inputs, with_mask)
    res = None
    for attempt in range(3):
        try:
            res = run_bass_kernel_spmd(nc, in_maps,
                                       core_ids=list(range(NCORES)))
            break
        except Exception:
            # transient NRT/axon device errors occasionally wedge a run;
            # back off and retry on a fresh execution
            if attempt == 2:
                raise
            import time
            time.sleep(15)
    LAST_RESULTS = res

    out = np.empty((B, H, S, D), np.float32)
    attn = np.empty((B, H, S, S), np.float32)
    for c in range(NCORES):
        b, h = c // H, c % H
        out[b, h] = res.results[c]["out"]
        attn[b, h] = res.results[c]["attn"]
    return out, attn


# revision 21
# speedup vs baseline: 1.0589x; 1.0589x over previous
"""Trainium2 Bass kernel for nn_Attention_33457795236557.

Math (B,H,S,D,HID = 2,4,512,16,32):
  qp = q@Wq+bq ; kp = k@Wk+bk ; vp = v@Wv+bv
  term1[i,j] = relu(qp_i@A + kp_j@Bm + b1) @ N2        (A=N1[:D], Bm=N1[D:])
  term2[i,j] = relu(kp_j@A + qp_i@Bm + b1) @ N2        (symmetrized, swapaxes)
  logits = term1 + term2 (+2*b2, const -> softmax-invariant) + mask*-1e9
  attn = softmax(logits, axis=-1) ; out = (attn @ vp) @ Wo + bo

Sharding: data-parallel over (b,h) -> 8 cores, one (b,h) pair each.

Device strategy per core (S=512 queries, tiled 4x128):
  Host folds all weight products:  hqA = q_aug @ CQ1, hkB' = k_aug @ CB1 (+b1),
  etc., with q_aug = [q | 1].  For each 128-query tile, queries are processed
  in blocks of 4; a block's relu input lives in a [128, 512] SBUF tile with
  partition p = i_local*32 + f (4 queries x 32 hidden), built by one
  DVE tensor_scalar (or ACT relu) op:  R = relu(HkRep + hq_col).
  HkRep[p, j] = hk'[j, f(p)] is precomputed once per core via one PE matmul.
  The hidden contraction R -> logits uses one fp32r PE matmul per relu tile
  with a zero-padded shifted block-diagonal N2 weight (lhsT [128,128], only
  columns 4b'..4b'+3 nonzero), all 64 matmuls of a query tile accumulating
  into one full [128, 512] PSUM bank (PE time is stream-column bound, so the
  zero padding is free).
  Softmax: ACT exp straight off PSUM logits (|logit| <= ~6, so the max
  subtraction is skipped -- softmax is shift invariant and fp32 exp cannot
  overflow) with accum_out producing the row sums; DVE reciprocal; DVE
  tensor_scalar normalize.  Output: PE transposes of the unnormalized attn,
  then attn @ (v_aug @ [Wv@Wo; bv@Wo+bo]) accumulated over 4 key chunks on
  PE, scaled by 1/rowsum at the end (bias terms fold through because
  normalized attn rows sum to 1).
"""

from contextlib import ExitStack

import numpy as np

import concourse.tile as tile
from concourse import bacc, mybir
from concourse.bass_utils import run_bass_kernel_spmd

B, H, S, D, HID = 2, 4, 512, 16, 32
NCORES = 8
F32 = mybir.dt.float32
F32R = mybir.dt.float32r

# Fraction of relu tiles routed to the scalar (ACT) engine: indices with
# (ridx % ACT_DEN) < ACT_MOD go to ACT, rest to DVE.
ACT_MOD = 1
ACT_DEN = 3
# engine toggles for small ops (tuned via TimelineSim)
HQCOL_ENG = "vector"   # gpsimd | vector
AT_ENG = "scalar"      # scalar | vector
HKREP_ENG = "scalar"   # scalar | vector
# dtype of the relu tiles + N2 weights feeding the PE contraction:
# "f32r" (full precision path, HW err ~1.6e-4) or "bf16" (faster DVE 4x mode)
R_DTYPE = "f32r"

LAST_RESULTS = None


def _build_program(with_mask: bool):
    nc = bacc.Bacc("TRN2", target_bir_lowering=False, debug=False,
                   enable_asserts=False)

    # Column layout of the packed f32 constants tensor [17, 1872]
    # (one hot DMA; ident and the fp32r N2 block-diagonals ship separately).
    COLS = dict(qT=(0, 512), kT=(512, 1024), vT=(1024, 1536),
                cq1=(1536, 1568), cq2=(1568, 1600),
                cb1=(1600, 1728), cb2=(1728, 1856), wvo=(1856, 1872))
    W = 1872
    # fp32r so the prework matmuls run at 1 cycle/row instead of fp32's 4
    # (input rounding costs ~1e-4 relative, same order as the contraction)
    packed = nc.dram_tensor("packed", [D + 1, W], F32R,
                            kind="ExternalInput").ap()
    ident_d = nc.dram_tensor("ident", [128, 128], F32,
                             kind="ExternalInput").ap()
    rdt = F32R if R_DTYPE == "f32r" else mybir.dt.bfloat16
    n2shift = nc.dram_tensor("n2shift", [128, 32 * 128], rdt,
                             kind="ExternalInput").ap()
    if with_mask:
        maskeff = nc.dram_tensor("maskeff", [S, S], F32,
                                 kind="ExternalInput").ap()

    out_d = nc.dram_tensor("out", [S, D], F32, kind="ExternalOutput").ap()
    attn_d = nc.dram_tensor("attn", [S, S], F32, kind="ExternalOutput").ap()

    with tile.TileContext(nc) as tc, ExitStack() as ctx:
        consts = ctx.enter_context(tc.tile_pool(name="consts", bufs=1))
        hkrep_p = ctx.enter_context(tc.tile_pool(name="hkrep", bufs=1))
        hqt_p = ctx.enter_context(tc.tile_pool(name="hqt", bufs=1))
        hqcol_p = ctx.enter_context(tc.tile_pool(name="hqcol", bufs=3))
        r_p = ctx.enter_context(tc.tile_pool(name="r", bufs=16))
        attn_p = ctx.enter_context(tc.tile_pool(name="attn", bufs=2))
        attnT_p = ctx.enter_context(tc.tile_pool(name="attnT", bufs=5))
        stats_p = ctx.enter_context(tc.tile_pool(name="stats", bufs=4))
        outsb_p = ctx.enter_context(tc.tile_pool(name="outsb", bufs=2))
        vpo_p = ctx.enter_context(tc.tile_pool(name="vpo", bufs=1))

        pre_ps = ctx.enter_context(
            tc.tile_pool(name="pre_ps", bufs=2, space="PSUM"))
        logit_ps = ctx.enter_context(
            tc.tile_pool(name="logit_ps", bufs=3, space="PSUM"))
        t_ps = ctx.enter_context(
            tc.tile_pool(name="t_ps", bufs=2, space="PSUM"))
        o_ps = ctx.enter_context(
            tc.tile_pool(name="o_ps", bufs=1, space="PSUM"))

        # ---- load constants/inputs ----
        big = consts.tile([D + 1, W], F32R, tag="packed")
        nc.sync.dma_start(big[:], packed[:])
        # N2 block-diagonals in 4 chunks so the first contraction matmuls
        # don't wait on the full 2MB transfer.
        n2_chunks = []
        for ch in range(4):
            t = consts.tile([128, 8 * 128], rdt, tag=f"n2_{ch}")
            nc.sync.dma_start(t[:], n2shift[:, 1024 * ch:1024 * (ch + 1)])
            n2_chunks.append(t)
        ident_s = consts.tile([128, 128], F32, tag="ident")
        nc.sync.dma_start(ident_s[:], ident_d[:])

        def cview(name):
            lo, hi = COLS[name]
            return big[:, lo:hi]

        qT_s, kT_s, vT_s = cview("qT"), cview("kT"), cview("vT")
        cq_s = [cview("cq1"), cview("cq2")]
        cb_s = [cview("cb1"), cview("cb2")]
        wvo_s = cview("wvo")

        # ---- prework (hkrep first: it gates every relu tile) ----
        # HkRep[term][p, j] = hk'[j, f(p)] replicated over i_local: [128, 512]
        hkrep_s = []
        for term in range(2):
            ps = pre_ps.tile([128, S], F32, tag="pre")
            nc.tensor.matmul(ps[:], cb_s[term][:], kT_s[:],
                             start=True, stop=True, skip_group_check=True)
            t = hkrep_p.tile([128, S], rdt if R_DTYPE == "bf16" else F32,
                             tag=f"hkrep{term}")
            if HKREP_ENG == "scalar":
                nc.scalar.copy(t[:], ps[:])
            else:
                nc.vector.tensor_copy(t[:], ps[:])
            hkrep_s.append(t)

        # hqT[term] = (q-side hidden)^T : [32, 512]
        hqt_s = []
        for term in range(2):
            ps = pre_ps.tile([HID, S], F32, tag="pre")
            nc.tensor.matmul(ps[:], cq_s[term][:], qT_s[:],
                             start=True, stop=True, skip_group_check=True)
            t = hqt_p.tile([HID, S], F32, tag=f"hqt{term}")
            nc.vector.tensor_copy(t[:], ps[:])
            hqt_s.append(t)

        # vpo' = v_aug @ [WvWo; bvWo+bo], per 128-key chunk (needed only by
        # the first out-path ~25us in)
        vpo_s = vpo_p.tile([128, 4 * D], F32)
        for c in range(4):
            ps = pre_ps.tile([128, D], F32, tag="pre")
            nc.tensor.matmul(ps[:], vT_s[:, 128 * c:128 * (c + 1)], wvo_s[:],
                             start=True, stop=True, skip_group_check=True)
            nc.vector.tensor_copy(vpo_s[:, D * c:D * (c + 1)], ps[:])

        # ---- main loop over 4 query tiles (software-pipelined emission:
        # tile tq's relu+matmul stream first, then tile tq-1's softmax/out
        # chain, so the ACT exp never convoys the next tile's relu work) ----
        def emit_softmax_out(tq, logits):
            if with_mask:
                msk = attn_p.tile([128, S], F32, tag="mask")
                nc.sync.dma_start(
                    msk[:], maskeff[128 * tq:128 * (tq + 1), :])
                masked = attn_p.tile([128, S], F32, tag="masked")
                nc.vector.tensor_add(out=masked[:], in0=logits[:],
                                     in1=msk[:])
                lsrc = masked
            else:
                lsrc = logits
            attn_un = attn_p.tile([128, S], F32, tag="attn_un")
            sumexp = stats_p.tile([128, 1], F32, tag="sumexp")
            nc.scalar.activation(attn_un[:], lsrc[:],
                                 mybir.ActivationFunctionType.Exp,
                                 bias=0.0, scale=1.0,
                                 accum_out=sumexp[:])
            rinv = stats_p.tile([128, 1], F32, tag="rinv")
            nc.vector.reciprocal(rinv[:], sumexp[:])
            attn_n = attn_p.tile([128, S], F32, tag="attn_n")
            nc.vector.tensor_scalar_mul(out=attn_n[:], in0=attn_un[:],
                                        scalar1=rinv[:])
            nc.sync.dma_start(attn_d[128 * tq:128 * (tq + 1), :], attn_n[:])

            # out = (attn @ vpo') * rinv
            ops = o_ps.tile([128, D], F32)
            for c in range(4):
                tp = t_ps.tile([128, 128], F32)
                nc.tensor.matmul(tp[:], attn_un[:, 128 * c:128 * (c + 1)],
                                 ident_s[:], is_transpose=True,
                                 skip_group_check=True)
                aT = attnT_p.tile([128, 128], F32, tag="aT")
                if AT_ENG == "scalar":
                    nc.scalar.copy(aT[:], tp[:])
                else:
                    nc.vector.tensor_copy(aT[:], tp[:])
                nc.tensor.matmul(ops[:], aT[:], vpo_s[:, D * c:D * (c + 1)],
                                 start=(c == 0), stop=(c == 3),
                                 skip_group_check=True)
            osb = outsb_p.tile([128, D], F32)
            nc.scalar.mul(osb[:], ops[:], rinv[:])
            nc.sync.dma_start(out_d[128 * tq:128 * (tq + 1), :], osb[:])

        ridx = 0
        pending = None
        for tq in range(4):
            # hq columns for this tile: [128, 32]; col b' holds
            # hq[128*tq + 4*b' + i_l, f] at partition p = i_l*32 + f
            hqcol = []
            for term in range(2):
                hc = hqcol_p.tile([128, 32], F32, tag=f"hqcol{term}")
                src3 = hqt_s[term][:, 128 * tq:128 * (tq + 1)].rearrange(
                    "p (b il) -> p b il", il=4)
                cp = (nc.gpsimd.tensor_copy if HQCOL_ENG == "gpsimd"
                      else nc.vector.tensor_copy)
                for il in range(4):
                    cp(hc[32 * il:32 * (il + 1), :], src3[:, :, il])
                hqcol.append(hc)

            logits = logit_ps.tile([128, S], F32)
            for bprime in range(32):
                for term in range(2):
                    r = r_p.tile([128, S], rdt, tag="r")
                    col = hqcol[term][:, bprime:bprime + 1]
                    if ((ridx * ACT_MOD) % ACT_DEN) < ACT_MOD:
                        nc.scalar.activation(
                            r[:], hkrep_s[term][:],
                            mybir.ActivationFunctionType.Relu,
                            bias=col, scale=1.0)
                    else:
                        nc.vector.tensor_scalar(
                            out=r[:], in0=hkrep_s[term][:],
                            scalar1=col, scalar2=0.0,
                            op0=mybir.AluOpType.add,
                            op1=mybir.AluOpType.max)
                    ridx += 1
                    nc.tensor.matmul(
                        logits[:],
                        n2_chunks[bprime // 8][:, 128 * (bprime % 8):
                                               128 * (bprime % 8 + 1)],
                        r[:],
                        start=(bprime == 0 and term == 0),
                        stop=(bprime == 31 and term == 1),
                        skip_group_check=True)

            if pending is not None:
                emit_softmax_out(*pending)
            pending = (tq, logits)
        emit_softmax_out(*pending)

    nc.compile()
    return nc


def _host_fold(inputs):
    """Fold weights on host; returns dict of per-core-constant arrays."""
    f = lambda x: np.asarray(x, dtype=np.float32)
    Wq, bq = f(inputs["Wq"]), f(inputs["bq"])
    Wk, bk = f(inputs["Wk"]), f(inputs["bk"])
    Wv, bv = f(inputs["Wv"]), f(inputs["bv"])
    Wo, bo = f(inputs["Wo"]), f(inputs["bo"])
    N1, b1 = f(inputs["N1"]), f(inputs["b1"])
    N2 = f(inputs["N2"])
    A, Bm = N1[:D], N1[D:]

    def aug(W, b):
        return np.vstack([W, b[None, :]]).astype(np.float32)

    cq1 = aug(Wq @ A, bq @ A)                    # term1 q-side  [17, 32]
    cq2 = aug(Wq @ Bm, bq @ Bm)                  # term2 q-side
    cb1 = np.tile(aug(Wk @ Bm, bk @ Bm + b1), (1, 4))   # term1 k-side [17,128]
    cb2 = np.tile(aug(Wk @ A, bk @ A + b1), (1, 4))     # term2 k-side
    wvo = aug(Wv @ Wo, bv @ Wo + bo)             # [17, 16]

    n2shift = np.zeros((32, 128, 128), np.float32)
    n2f = N2[:, 0]
    for bp in range(32):
        for il in range(4):
            for ff in range(HID):
                n2shift[bp, il * 32 + ff, 4 * bp + il] = n2f[ff]

    return dict(cq1=cq1, cq2=cq2, cb1=np.ascontiguousarray(cb1),
                cb2=np.ascontiguousarray(cb2), wvo=wvo,
                n2shift=np.ascontiguousarray(
                    n2shift.transpose(1, 0, 2).reshape(128, 32 * 128)),
                ident=np.eye(128, dtype=np.float32))


def _make_in_maps(inputs, with_mask):
    consts = _host_fold(inputs)
    q = np.asarray(inputs["q"], dtype=np.float32)
    k = np.asarray(inputs["k"], dtype=np.float32)
    v = np.asarray(inputs["v"], dtype=np.float32)
    mask = np.asarray(inputs["mask"], dtype=np.float32)
    b2 = np.asarray(inputs["b2"], dtype=np.float32)

    ones = np.ones((1, S), np.float32)
    W = 1872
    base = np.zeros((17, W), np.float32)
    base[:, 1536:1568] = consts["cq1"]
    base[:, 1568:1600] = consts["cq2"]
    base[:, 1600:1728] = consts["cb1"]
    base[:, 1728:1856] = consts["cb2"]
    base[:, 1856:1872] = consts["wvo"]

    in_maps = []
    for c in range(NCORES):
        b, h = c // H, c % H
        p = base.copy()
        p[:, 0:512] = np.vstack([q[b, h].T, ones])
        p[:, 512:1024] = np.vstack([k[b, h].T, ones])
        p[:, 1024:1536] = np.vstack([v[b, h].T, ones])
        n2 = consts["n2shift"]
        if R_DTYPE == "bf16":
            import ml_dtypes
            n2 = n2.astype(ml_dtypes.bfloat16)
        m = {"packed": p, "n2shift": n2, "ident": consts["ident"]}
        if with_mask:
            m["maskeff"] = np.ascontiguousarray(
                mask[b, 0] * np.float32(-1e9) + 2.0 * b2[0])
        in_maps.append(m)
    return in_maps


_PROGRAM_CACHE = {}


def kernel(**inputs):
    global LAST_RESULTS
    mask = np.asarray(inputs["mask"], dtype=np.float32)
    with_mask = bool(np.any(mask))

    if with_mask not in _PROGRAM_CACHE:
        _PROGRAM_CACHE[with_mask] = _build_program(with_mask)
    nc = _PROGRAM_CACHE[with_mask]

    in_maps = _make_in_maps(inputs, with_mask)
    res = None
    for attempt in range(3):
        try:
            res = run_bass_kernel_spmd(nc, in_maps,
                                       core_ids=list(range(NCORES)))
            break
        except Exception:
            # transient NRT/axon device errors occasionally wedge a run;
            # back off and retry on a fresh execution
            if attempt == 2:
                raise
            import time
            time.sleep(15)
    LAST_RESULTS = res

    out = np.empty((B, H, S, D), np.float32)
    attn = np.empty((B, H, S, S), np.float32)
    for c in range(NCORES):
        b, h = c // H, c % H
        out[b, h] = res.results[c]["out"]
        attn[b, h] = res.results[c]["attn"]
    return out, attn
